# revision 41
# baseline (speedup 1.0000x reference)
"""Trainium2 Bass kernel for nn_AttentionHead (B=8, S=2048, H=1024, D=64).

Sharding: data-parallel over batch -- one batch element per NeuronCore,
8 cores, no collectives.  The kernel is DMA-bound (one serial ~360 GB/s
HBM stream per core in the cost model), so the design minimizes bytes
moved and keeps the DMA queue packed, while pacing every compute engine
below the DMA stream time:

  - query/key/value stream in pre-transposed as [H, S] fp16 slabs; the
    relative bias streams as [Sk, Sq] fp8 (e3m4: its ~1.8e-2 relative
    quantization becomes ~2e-3 logit noise after the 1/sqrt(d) scale,
    i.e. ~0.25% output error against the 2% gate);
  - kT/qT [64, S] project on PE in 512-column slabs; v projects
    directly into [s, d] tiles (full 128-partition outputs, half the PE
    cycles of a transposed projection, no PE transposes), with the key
    mask folded in multiplicatively;
  - the two 1024-column query blocks are processed INTERLEAVED, paced
    by the bias-tile DMAs, so the ACT engine's exp stream (the #2
    engine) never bunches at the end;
  - scoresT[sk, sq] = kT-slice.T @ qT accumulates in PSUM; the bias is
    added by an fp8-identity matmul on PE or by the vector engine
    (alternating, to balance the two);
  - exp on ACT (scale=1/sqrt(d), no max-subtraction: logits ~N(0,1),
    overflow-impossible) writes fp16 att tiles;
  - AV runs transposed-back: out[sq, d] accumulates over sk with att
    128x128 slices as the stationary operand and v [128, 64] as moving
    data (64 moving columns instead of 512), and the softmax
    denominator accumulates beside it from 1-column mask matmuls;
  - k/v linear biases are algebraically removed from the device (bk
    shifts every logit of a query equally -> softmax-invariant; bv adds
    bv*den to the numerator -> host adds bv after the division); the
    final division and [s-tile] re-assembly happen on the host.

PSUM (8 banks): 2x2 score tiles + 2x1 AV accumulators + 1 denominator
+ 1 projection slab.
"""

import os
from contextlib import ExitStack

import numpy as np

import concourse.bass as bass
import concourse.tile as tile
from concourse import bacc, mybir
from concourse.bass_utils import run_bass_kernel_spmd
from concourse.masks import make_identity

B, S, H, D = 8, 2048, 1024, 64
N_CORES = 8
FP = mybir.dt.float32
F16 = mybir.dt.float16
F8 = mybir.dt.float8e3

SQ_BLK = 1024           # sq columns per outer block
NT = S // SQ_BLK        # outer blocks
NK = S // 128           # sk tiles
NH = H // 128           # hidden chunks
NSQT = SQ_BLK // 128    # sq tiles per block

# engine that adds the bias, rotating per attn half: 'pe' or 've';
# 've' first within each 8-half group so DVE is free near boundaries
# when a projection copy (which gates the next group) hits its queue
_PAT = os.environ.get(
    "KERNEL_BIAS_PAT", "pe,ve,pe,ve,pe,ve,ve,pe").split(",")


def _np_f8():
    import ml_dtypes

    return ml_dtypes.float8_e3m4


def build_bass():
    nc = bacc.Bacc("TRN2", target_bir_lowering=False, debug=False,
                   num_devices=N_CORES)

    xqT = nc.dram_tensor("xqT", [H, S], F16, kind="ExternalInput").ap()
    xkT = nc.dram_tensor("xkT", [H, S], F16, kind="ExternalInput").ap()
    xvT = nc.dram_tensor("xvT", [H, S], F16, kind="ExternalInput").ap()
    biasT = nc.dram_tensor("biasT", [S, S], F8, kind="ExternalInput").ap()
    # packed small constants: cols 0:NK mask, col NK bq
    cpk = nc.dram_tensor("cpk", [128, NK + 1], FP, kind="ExternalInput").ap()
    # weights pre-laid out as the SBUF image: [128, NH*D]
    wqT = nc.dram_tensor("wqT", [128, NH * D], F16, kind="ExternalInput").ap()
    wkT = nc.dram_tensor("wkT", [128, NH * D], F16, kind="ExternalInput").ap()
    wvT = nc.dram_tensor("wvT", [128, NH * D], F16, kind="ExternalInput").ap()
    out_d = nc.dram_tensor("out", [NT, 128, NSQT * (D + 1)], F16,
                           kind="ExternalOutput").ap()

    with tile.TileContext(nc) as tc, ExitStack() as ctx:
        const = ctx.enter_context(tc.tile_pool(name="const", bufs=1))
        xslab = ctx.enter_context(tc.tile_pool(
            name="xslab", bufs=int(os.environ.get("KERNEL_XBUFS", "4"))))
        bias_in = ctx.enter_context(tc.tile_pool(
            name="bias_in", bufs=int(os.environ.get("KERNEL_BIASBUFS", "6"))))
        att_pool = ctx.enter_context(tc.tile_pool(
            name="att", bufs=int(os.environ.get("KERNEL_ATTBUFS", "22"))))
        avsb_pool = ctx.enter_context(tc.tile_pool(name="avsb", bufs=2))
        # one shared 5-slot ring of 1-bank [128,512] PSUM tiles serves score
        # halves, projection slabs and v tiles; + 2 AV accumulators + den
        ps_wk = ctx.enter_context(tc.tile_pool(
            name="ps_wk", bufs=int(os.environ.get("KERNEL_WKBUFS", "5")),
            space="PSUM"))
        ps_av = ctx.enter_context(tc.tile_pool(name="ps_av", bufs=2,
                                               space="PSUM"))
        ps_dn = ctx.enter_context(tc.tile_pool(name="ps_dn", bufs=1,
                                               space="PSUM"))

        # ---- constants (wk/xk0 issue first; see weave below) ----
        w_sb = {}

        def fetch_w(name, wT):
            w = const.tile([128, NH, D], F16, tag=f"w{name}", name=f"w{name}")
            nc.sync.dma_start(out=w.rearrange("p t d -> p (t d)"), in_=wT)
            w_sb[name] = w

        def fetch_consts():
            cp = const.tile([128, NK + 1], FP, tag="cpk", name="cpk")
            nc.sync.dma_start(out=cp, in_=cpk)
            ident = const.tile([128, 128], FP, tag="ident", name="ident")
            make_identity(nc, ident)
            ident8 = const.tile([128, 128], F8, tag="ident8", name="ident8")
            nc.vector.tensor_copy(out=ident8, in_=ident)
            mask16 = const.tile([128, NK], F16, tag="mask16", name="mask16")
            nc.vector.tensor_copy(out=mask16, in_=cp[:, 0:NK])
            return cp, ident8, mask16

        xT_of = {"k": xkT, "q": xqT, "v": xvT}

        def fetch_x(name, j, split=False):
            x = xslab.tile([128, NH, 512], F16, tag="x", name=f"x_{name}_{j}")
            src = xT_of[name][:, j * 512:(j + 1) * 512].rearrange(
                "(h p) c -> p h c", p=128)
            if split:
                # two half-slab DMAs: projection chunks 0..3 start (via
                # subtile deps) while the second half is still in flight
                nc.sync.dma_start(out=x[:, 0:4, :], in_=src[:, 0:4, :])
                nc.sync.dma_start(out=x[:, 4:8, :], in_=src[:, 4:8, :])
            else:
                nc.sync.dma_start(out=x, in_=src)
            return x

        bias_of = {}

        def fetch_bias(nt, sk0, n=4):
            bt = bias_in.tile([128, 4, SQ_BLK], F8, tag="bias",
                              name=f"bias_{nt}_{sk0}")
            nc.sync.dma_start(
                out=bt[:, 0:n, :],
                in_=biasT[sk0 * 128:(sk0 + n) * 128,
                          nt * SQ_BLK:(nt + 1) * SQ_BLK].rearrange(
                    "(j p) c -> p j c", p=128))
            for i in range(n):
                bias_of[(nt, sk0 + i)] = bt[:, i, :]

        # the weave below calls these in DMA-stream order
        kT_sb = const.tile([D, S], F16, tag="kT", name="kT")
        qT_sb = const.tile([D, S], F16, tag="qT", name="qT")
        v_sb = const.tile([128, NK, D], F16, tag="v_sb", name="v_sb")

        def proj_kq(name, dst, j, x, b=None, split_copy=False):
            if split_copy:
                # four independent [64,128] accumulations + per-tile copies:
                # the group's first score matmul is gated by one 128-column
                # tile instead of the whole 512-column slab
                for t in range(4):
                    ps = ps_wk.tile([D, 128], FP, tag="wk",
                                    name=f"ps_{name}_{j}_{t}")
                    for h in range(NH):
                        nc.tensor.matmul(
                            ps, lhsT=w_sb[name][:, h, :],
                            rhs=x[:, h, t * 128:(t + 1) * 128],
                            start=(h == 0), stop=(h == NH - 1))
                    c0 = j * 512 + t * 128
                    nc.vector.tensor_copy(out=dst[:, c0:c0 + 128], in_=ps)
                return
            ps = ps_wk.tile([D, 512], FP, tag="wk", name=f"ps_{name}_{j}")
            for h in range(NH):
                nc.tensor.matmul(ps, lhsT=w_sb[name][:, h, :],
                                 rhs=x[:, h, :],
                                 start=(h == 0), stop=(h == NH - 1))
            dcols = dst[:, j * 512:(j + 1) * 512]
            if b is None:
                nc.vector.tensor_copy(out=dcols, in_=ps)
            else:
                nc.vector.tensor_scalar_add(out=dcols, in0=ps, scalar1=b)

        def proj_v(j, x, cp):
            for t in range(4):
                sk = 4 * j + t
                ps = ps_wk.tile([128, D], FP, tag="wk", name=f"ps_v_{sk}")
                for h in range(NH):
                    nc.tensor.matmul(ps, lhsT=x[:, h, t * 128:(t + 1) * 128],
                                     rhs=w_sb["v"][:, h, :],
                                     start=(h == 0), stop=(h == NH - 1))
                nc.vector.tensor_scalar_mul(out=v_sb[:, sk, :], in0=ps,
                                            scalar1=cp[:, sk:sk + 1])

        # ---- attention ----
        inv_sqrt_d = 1.0 / np.sqrt(float(D))
        av_of = {}
        den = ps_dn.tile([128, NT * NSQT], FP, tag="den", name="den")
        state = {"tile_idx": 0, "den_idx": 0, "v_ready": -1}
        pending_av = []

        def issue_av(att, nt, sk, half, mask16):
            av = av_of[nt]
            for t in range(half * 4, half * 4 + 4):
                nc.tensor.matmul(av[:, t, :],
                                 lhsT=att[:, t * 128:(t + 1) * 128],
                                 rhs=v_sb[:, sk, :],
                                 start=(sk == 0 and t == 0),
                                 stop=(sk == NK - 1 and t == NSQT - 1))
            for t in range(half * 4, half * 4 + 4):
                i = state["den_idx"]
                state["den_idx"] = i + 1
                nc.tensor.matmul(den[:, nt * NSQT + t:nt * NSQT + t + 1],
                                 lhsT=att[:, t * 128:(t + 1) * 128],
                                 rhs=mask16[:, sk:sk + 1],
                                 start=(i == 0),
                                 stop=(i == NT * NK * NSQT - 1))

        def flush_av(mask16, keep=2, n_max=2, only_nt=None):
            # n_max spreads AV backlog bursts (waiting on a v slab) across
            # several attn halves so they don't clog the PE queue at once;
            # only_nt avoids parking AVs whose exp hasn't run (in-order PE
            # head-of-line) when finishing one block while the other still
            # streams
            issued = 0
            idx = 0
            while issued < n_max and idx < len(pending_av) - keep:
                att, nt, sk, half = pending_av[idx]
                if sk > state["v_ready"] or (only_nt is not None
                                             and nt != only_nt):
                    idx += 1
                    continue
                pending_av.pop(idx)
                issue_av(att, nt, sk, half, mask16)
                issued += 1

        att_of = {}

        def attn_half(nt, sk, i, ident8, mask16):
            # independent 512-column halves: 1-bank PSUM slots (deep
            # pipeline) and finer exp/AV dependencies
            sq0 = nt * SQ_BLK
            bias_t = bias_of[(nt, sk)]
            if (nt, sk) not in att_of:
                att_of[(nt, sk)] = att_pool.tile(
                    [128, SQ_BLK], F16, tag="att", name=f"att_{nt}_{sk}")
            att = att_of[(nt, sk)]
            cols = slice(i * 512, (i + 1) * 512)
            sc = ps_wk.tile([128, 512], FP, tag="wk",
                            name=f"sc_{nt}_{sk}_{i}")
            eng = _PAT[state["tile_idx"] % len(_PAT)]
            state["tile_idx"] += 1
            nc.tensor.matmul(
                sc,
                lhsT=kT_sb[:, sk * 128:(sk + 1) * 128],
                rhs=qT_sb[:, sq0 + i * 512:sq0 + (i + 1) * 512],
                start=True, stop=(eng != "pe"))
            if eng == "pe":
                nc.tensor.matmul(sc, lhsT=ident8, rhs=bias_t[:, cols],
                                 start=False, stop=True)
            else:
                nc.vector.tensor_add(out=sc, in0=sc, in1=bias_t[:, cols])
            nc.scalar.activation(out=att[:, cols], in_=sc,
                                 func=mybir.ActivationFunctionType.Exp,
                                 scale=inv_sqrt_d)
            # AV trails so the in-order PE queue never head-of-line
            # blocks on an exp result
            pending_av.append((att, nt, sk, i))
            flush_av(mask16, keep=state.get("flush_keep", 2))

        def attn(nt, sk, ident8, mask16):
            attn_half(nt, sk, 0, ident8, mask16)
            attn_half(nt, sk, 1, ident8, mask16)

        def start_nt(nt):
            av_of[nt] = ps_av.tile([128, NSQT, D], FP, tag="av",
                                   name=f"av_{nt}")

        def finish_nt(nt, mask16):
            flush_av(mask16, keep=0, n_max=10000, only_nt=nt)
            avs = avsb_pool.tile([128, NSQT, D + 1], F16, tag="avs",
                                 name=f"avs_{nt}")
            nc.vector.tensor_copy(out=avs[:, :, 0:D], in_=av_of[nt])
            nc.vector.tensor_copy(
                out=avs[:, :, D],
                in_=den[:, nt * NSQT:(nt + 1) * NSQT])
            nc.sync.dma_start(
                out=out_d[nt], in_=avs.rearrange("p t d -> p (t d)"))

        # ---- the woven stream (DMA order is the schedule) ----
        # All x slabs are fetched as two half-slab DMAs and all bias groups
        # as 2-tile fetches, interpolated so the exp-work supply tracks the
        # ACT engine's consumption (~2.45us of exp work per 2-tile bias)
        # with no dry spells after the pipeline fills.
        fetch_w("k", wkT)
        xk0 = fetch_x("k", 0, split=True)
        fetch_w("q", wqT)
        cp, ident8, mask16 = fetch_consts()
        bq_col = cp[0:D, NK:NK + 1]
        wk_flat = w_sb["k"].rearrange("p t d -> p (t d)")
        n_warm = int(os.environ.get("KERNEL_WARMUP", "9"))
        if n_warm:
            warm = ps_wk.tile([D, 512], FP, tag="wk", name="warm")
            for i in range(n_warm):
                nc.tensor.matmul(warm, lhsT=w_sb["k"][:, 0, :], rhs=wk_flat,
                                 start=True, stop=True)
        proj_kq("k", kT_sb, 0, xk0, split_copy=True)
        xq0 = fetch_x("q", 0, split=True)
        proj_kq("q", qT_sb, 0, xq0, bq_col)
        fetch_bias(0, 0, n=2)
        fetch_bias(0, 2, n=2)
        start_nt(0)
        start_nt(1)
        for sk in range(0, 4):
            attn_half(0, sk, 0, ident8, mask16)
        xq1 = fetch_x("q", 1, split=True)
        proj_kq("q", qT_sb, 1, xq1, bq_col)
        for sk in range(0, 4):
            attn_half(0, sk, 1, ident8, mask16)
        xk1 = fetch_x("k", 1, split=True)
        proj_kq("k", kT_sb, 1, xk1, split_copy=True)
        fetch_bias(0, 4, n=2)
        attn(0, 4, ident8, mask16)
        attn(0, 5, ident8, mask16)
        fetch_bias(0, 6, n=2)
        attn(0, 6, ident8, mask16)
        attn(0, 7, ident8, mask16)
        xq2 = fetch_x("q", 2, split=True)
        proj_kq("q", qT_sb, 2, xq2, bq_col)
        fetch_bias(1, 0, n=2)
        attn_half(1, 0, 0, ident8, mask16)
        attn_half(1, 1, 0, ident8, mask16)
        fetch_bias(1, 2, n=2)
        attn_half(1, 2, 0, ident8, mask16)
        attn_half(1, 3, 0, ident8, mask16)
        xq3 = fetch_x("q", 3, split=True)
        proj_kq("q", qT_sb, 3, xq3, bq_col)
        for sk in range(0, 4):
            attn_half(1, sk, 1, ident8, mask16)
        fetch_bias(1, 4, n=2)
        attn(1, 4, ident8, mask16)
        attn(1, 5, ident8, mask16)
        fetch_bias(1, 6, n=2)
        attn(1, 6, ident8, mask16)
        attn(1, 7, ident8, mask16)
        xk2 = fetch_x("k", 2, split=True)
        proj_kq("k", kT_sb, 2, xk2, split_copy=True)
        fetch_bias(0, 8, n=2)
        attn(0, 8, ident8, mask16)
        attn(0, 9, ident8, mask16)
        fetch_bias(0, 10, n=2)
        attn(0, 10, ident8, mask16)
        attn(0, 11, ident8, mask16)
        fetch_w("v", wvT)
        xv0 = fetch_x("v", 0, split=True)
        proj_v(0, xv0, cp)
        state["v_ready"] = 3
        flush_av(mask16, keep=2, n_max=6)
        fetch_bias(1, 8, n=2)
        attn(1, 8, ident8, mask16)
        attn(1, 9, ident8, mask16)
        fetch_bias(1, 10, n=2)
        attn(1, 10, ident8, mask16)
        attn(1, 11, ident8, mask16)
        xk3 = fetch_x("k", 3, split=True)
        proj_kq("k", kT_sb, 3, xk3, split_copy=True)
        fetch_bias(0, 12, n=2)
        attn(0, 12, ident8, mask16)
        attn(0, 13, ident8, mask16)
        fetch_bias(0, 14, n=2)
        attn(0, 14, ident8, mask16)
        attn(0, 15, ident8, mask16)
        xv1 = fetch_x("v", 1, split=True)
        proj_v(1, xv1, cp)
        state["v_ready"] = 7
        flush_av(mask16, keep=2, n_max=8)
        fetch_bias(1, 12, n=2)
        attn(1, 12, ident8, mask16)
        attn(1, 13, ident8, mask16)
        xv2 = fetch_x("v", 2, split=True)
        proj_v(2, xv2, cp)
        state["v_ready"] = 11
        flush_av(mask16, keep=2, n_max=8)
        xv3 = fetch_x("v", 3, split=True)
        proj_v(3, xv3, cp)
        state["v_ready"] = 15
        # drain nt0 completely and ship its output before the last two
        # attention tiles, so the final exp->AV->avs->out chain is short
        flush_av(mask16, keep=0, n_max=100, only_nt=0)
        finish_nt(0, mask16)
        state["flush_keep"] = 100  # last AVs drain in finish_nt(1)
        fetch_bias(1, 14, n=1)
        fetch_bias(1, 15, n=1)
        attn(1, 14, ident8, mask16)
        attn(1, 15, ident8, mask16)
        finish_nt(1, mask16)

    nc.compile()
    return nc


_NC = None


def _get_nc():
    global _NC
    if _NC is None:
        _NC = build_bass()
    return _NC


def _prep_core_inputs(b, query, key, value, relative_biases, mask,
                      Wq, bq, Wk, bk, Wv, bv):
    def wprep(W):
        # SBUF image [128, NH*D]: (p, t*D+d) = W.T[t*128+p, d]
        return np.ascontiguousarray(
            W.T.astype(np.float16).reshape(NH, 128, D).transpose(
                1, 0, 2).reshape(128, NH * D))

    cp = np.zeros((128, NK + 1), np.float32)
    cp[:, 0:NK] = mask[b].astype(np.float32).reshape(NK, 128).T
    cp[0:D, NK] = np.asarray(bq, np.float32)

    return {
        "xqT": np.ascontiguousarray(query[b].T.astype(np.float16)),
        "xkT": np.ascontiguousarray(key[b].T.astype(np.float16)),
        "xvT": np.ascontiguousarray(value[b].T.astype(np.float16)),
        "biasT": np.ascontiguousarray(
            relative_biases[b].T.astype(_np_f8())),
        "cpk": cp,
        "wqT": wprep(Wq),
        "wkT": wprep(Wk),
        "wvT": wprep(Wv),
    }


def kernel(query, key, value, relative_biases, mask, Wq, bq, Wk, bk, Wv, bv):
    query = np.asarray(query, np.float32)
    key = np.asarray(key, np.float32)
    value = np.asarray(value, np.float32)
    relative_biases = np.asarray(relative_biases, np.float32)
    mask = np.asarray(mask)
    Wq, Wk, Wv = (np.asarray(w, np.float32) for w in (Wq, Wk, Wv))
    bv_row = np.asarray(bv, np.float32).reshape(1, D)

    nc = _get_nc()
    in_maps = [
        _prep_core_inputs(b, query, key, value, relative_biases, mask,
                          Wq, bq, Wk, bk, Wv, bv)
        for b in range(B)
    ]
    res = run_bass_kernel_spmd(nc, in_maps, core_ids=list(range(N_CORES)))
    outs = []
    for i in range(N_CORES):
        o = np.asarray(res.results[i]["out"], dtype=np.float32)
        o = o.reshape(NT, 128, NSQT, D + 1)
        o = o[:, :, :, :D] / o[:, :, :, D:D + 1] + bv_row
        # out[s = nt*1024 + t*128 + p, d] = o[nt, p, t, d]
        outs.append(o.transpose(0, 2, 1, 3).reshape(S, D))
    return np.stack(outs, axis=0).astype(np.float32)


# revision 42
# speedup vs baseline: 1.0075x; 1.0075x over previous
"""Trainium2 Bass kernel for nn_AttentionHead (B=8, S=2048, H=1024, D=64).

Sharding: data-parallel over batch -- one batch element per NeuronCore,
8 cores, no collectives.  The kernel is DMA-bound (one serial ~360 GB/s
HBM stream per core in the cost model), so the design minimizes bytes
moved and keeps the DMA queue packed, while pacing every compute engine
below the DMA stream time:

  - query/key/value stream in pre-transposed as [H, S] fp16 slabs; the
    relative bias streams as [Sk, Sq] fp8 (e3m4: its ~1.8e-2 relative
    quantization becomes ~2e-3 logit noise after the 1/sqrt(d) scale,
    i.e. ~0.25% output error against the 2% gate);
  - kT/qT [64, S] project on PE in 512-column slabs; v projects
    directly into [s, d] tiles (full 128-partition outputs, half the PE
    cycles of a transposed projection, no PE transposes), with the key
    mask folded in multiplicatively;
  - the two 1024-column query blocks are processed INTERLEAVED, paced
    by the bias-tile DMAs, so the ACT engine's exp stream (the #2
    engine) never bunches at the end;
  - scoresT[sk, sq] = kT-slice.T @ qT accumulates in PSUM; the bias is
    added by an fp8-identity matmul on PE or by the vector engine
    (alternating, to balance the two);
  - exp on ACT (scale=1/sqrt(d), no max-subtraction: logits ~N(0,1),
    overflow-impossible) writes fp16 att tiles;
  - AV runs transposed-back: out[sq, d] accumulates over sk with att
    128x128 slices as the stationary operand and v [128, 64] as moving
    data (64 moving columns instead of 512), and the softmax
    denominator accumulates beside it from 1-column mask matmuls;
  - k/v linear biases are algebraically removed from the device (bk
    shifts every logit of a query equally -> softmax-invariant; bv adds
    bv*den to the numerator -> host adds bv after the division); the
    final division and [s-tile] re-assembly happen on the host.

PSUM (8 banks): 2x2 score tiles + 2x1 AV accumulators + 1 denominator
+ 1 projection slab.
"""

import os
from contextlib import ExitStack

import numpy as np

import concourse.bass as bass
import concourse.tile as tile
from concourse import bacc, mybir
from concourse.bass_utils import run_bass_kernel_spmd
from concourse.masks import make_identity

B, S, H, D = 8, 2048, 1024, 64
N_CORES = 8
FP = mybir.dt.float32
F16 = mybir.dt.float16
F8 = mybir.dt.float8e3

SQ_BLK = 1024           # sq columns per outer block
NT = S // SQ_BLK        # outer blocks
NK = S // 128           # sk tiles
NH = H // 128           # hidden chunks
NSQT = SQ_BLK // 128    # sq tiles per block

# engine that adds the bias, rotating per attn half: 'pe' or 've';
# 've' first within each 8-half group so DVE is free near boundaries
# when a projection copy (which gates the next group) hits its queue
_PAT = os.environ.get(
    "KERNEL_BIAS_PAT", "pe,ve,pe,ve,pe,ve,ve,pe").split(",")


def _np_f8():
    import ml_dtypes

    return ml_dtypes.float8_e3m4


def build_bass():
    nc = bacc.Bacc("TRN2", target_bir_lowering=False, debug=False,
                   num_devices=N_CORES)

    xqT = nc.dram_tensor("xqT", [H, S], F16, kind="ExternalInput").ap()
    xkT = nc.dram_tensor("xkT", [H, S], F16, kind="ExternalInput").ap()
    xvT = nc.dram_tensor("xvT", [H, S], F16, kind="ExternalInput").ap()
    biasT = nc.dram_tensor("biasT", [S, S], F8, kind="ExternalInput").ap()
    # packed small constants: cols 0:NK mask, col NK bq
    cpk = nc.dram_tensor("cpk", [128, NK + 1], FP, kind="ExternalInput").ap()
    # weights pre-laid out as the SBUF image: [128, NH*D]
    wqT = nc.dram_tensor("wqT", [128, NH * D], F16, kind="ExternalInput").ap()
    wkT = nc.dram_tensor("wkT", [128, NH * D], F16, kind="ExternalInput").ap()
    wvT = nc.dram_tensor("wvT", [128, NH * D], F16, kind="ExternalInput").ap()
    out_d = nc.dram_tensor("out", [NT, 128, NSQT * (D + 1)], F16,
                           kind="ExternalOutput").ap()

    with tile.TileContext(nc) as tc, ExitStack() as ctx:
        const = ctx.enter_context(tc.tile_pool(name="const", bufs=1))
        xslab = ctx.enter_context(tc.tile_pool(
            name="xslab", bufs=int(os.environ.get("KERNEL_XBUFS", "4"))))
        bias_in = ctx.enter_context(tc.tile_pool(
            name="bias_in", bufs=int(os.environ.get("KERNEL_BIASBUFS", "6"))))
        att_pool = ctx.enter_context(tc.tile_pool(
            name="att", bufs=int(os.environ.get("KERNEL_ATTBUFS", "22"))))
        avsb_pool = ctx.enter_context(tc.tile_pool(name="avsb", bufs=2))
        # one shared 5-slot ring of 1-bank [128,512] PSUM tiles serves score
        # halves, projection slabs and v tiles; + 2 AV accumulators + den
        ps_wk = ctx.enter_context(tc.tile_pool(
            name="ps_wk", bufs=int(os.environ.get("KERNEL_WKBUFS", "5")),
            space="PSUM"))
        ps_av = ctx.enter_context(tc.tile_pool(name="ps_av", bufs=2,
                                               space="PSUM"))
        ps_dn = ctx.enter_context(tc.tile_pool(name="ps_dn", bufs=1,
                                               space="PSUM"))

        # ---- constants (wk/xk0 issue first; see weave below) ----
        w_sb = {}

        def fetch_w(name, wT):
            w = const.tile([128, NH, D], F16, tag=f"w{name}", name=f"w{name}")
            nc.sync.dma_start(out=w.rearrange("p t d -> p (t d)"), in_=wT)
            w_sb[name] = w

        def fetch_consts():
            cp = const.tile([128, NK + 1], FP, tag="cpk", name="cpk")
            nc.sync.dma_start(out=cp, in_=cpk)
            ident = const.tile([128, 128], FP, tag="ident", name="ident")
            make_identity(nc, ident)
            ident8 = const.tile([128, 128], F8, tag="ident8", name="ident8")
            nc.vector.tensor_copy(out=ident8, in_=ident)
            mask16 = const.tile([128, NK], F16, tag="mask16", name="mask16")
            nc.vector.tensor_copy(out=mask16, in_=cp[:, 0:NK])
            return cp, ident8, mask16

        xT_of = {"k": xkT, "q": xqT, "v": xvT}

        def fetch_x(name, j, split=False):
            x = xslab.tile([128, NH, 512], F16, tag="x", name=f"x_{name}_{j}")
            src = xT_of[name][:, j * 512:(j + 1) * 512].rearrange(
                "(h p) c -> p h c", p=128)
            if split:
                # two half-slab DMAs: projection chunks 0..3 start (via
                # subtile deps) while the second half is still in flight
                nc.sync.dma_start(out=x[:, 0:4, :], in_=src[:, 0:4, :])
                nc.sync.dma_start(out=x[:, 4:8, :], in_=src[:, 4:8, :])
            else:
                nc.sync.dma_start(out=x, in_=src)
            return x

        bias_of = {}

        def fetch_bias(nt, sk0, n=4):
            bt = bias_in.tile([128, 4, SQ_BLK], F8, tag="bias",
                              name=f"bias_{nt}_{sk0}")
            nc.sync.dma_start(
                out=bt[:, 0:n, :],
                in_=biasT[sk0 * 128:(sk0 + n) * 128,
                          nt * SQ_BLK:(nt + 1) * SQ_BLK].rearrange(
                    "(j p) c -> p j c", p=128))
            for i in range(n):
                bias_of[(nt, sk0 + i)] = bt[:, i, :]

        # the weave below calls these in DMA-stream order
        kT_sb = const.tile([D, S], F16, tag="kT", name="kT")
        qT_sb = const.tile([D, S], F16, tag="qT", name="qT")
        v_sb = const.tile([128, NK, D], F16, tag="v_sb", name="v_sb")

        def proj_kq(name, dst, j, x, b=None, split_copy=False):
            if split_copy:
                # four independent [64,128] accumulations + per-tile copies:
                # the group's first score matmul is gated by one 128-column
                # tile instead of the whole 512-column slab
                for t in range(4):
                    ps = ps_wk.tile([D, 128], FP, tag="wk",
                                    name=f"ps_{name}_{j}_{t}")
                    for h in range(NH):
                        nc.tensor.matmul(
                            ps, lhsT=w_sb[name][:, h, :],
                            rhs=x[:, h, t * 128:(t + 1) * 128],
                            start=(h == 0), stop=(h == NH - 1))
                    c0 = j * 512 + t * 128
                    nc.vector.tensor_copy(out=dst[:, c0:c0 + 128], in_=ps)
                return
            ps = ps_wk.tile([D, 512], FP, tag="wk", name=f"ps_{name}_{j}")
            for h in range(NH):
                nc.tensor.matmul(ps, lhsT=w_sb[name][:, h, :],
                                 rhs=x[:, h, :],
                                 start=(h == 0), stop=(h == NH - 1))
            dcols = dst[:, j * 512:(j + 1) * 512]
            if b is None:
                nc.vector.tensor_copy(out=dcols, in_=ps)
            else:
                nc.vector.tensor_scalar_add(out=dcols, in0=ps, scalar1=b)

        def proj_v(j, x, cp):
            for t in range(4):
                sk = 4 * j + t
                ps = ps_wk.tile([128, D], FP, tag="wk", name=f"ps_v_{sk}")
                for h in range(NH):
                    nc.tensor.matmul(ps, lhsT=x[:, h, t * 128:(t + 1) * 128],
                                     rhs=w_sb["v"][:, h, :],
                                     start=(h == 0), stop=(h == NH - 1))
                nc.vector.tensor_scalar_mul(out=v_sb[:, sk, :], in0=ps,
                                            scalar1=cp[:, sk:sk + 1])

        # ---- attention ----
        inv_sqrt_d = 1.0 / np.sqrt(float(D))
        av_of = {}
        den = ps_dn.tile([128, NT * NSQT], FP, tag="den", name="den")
        state = {"tile_idx": 0, "den_idx": 0, "v_ready": -1}
        pending_av = []

        def issue_av(att, nt, sk, half, mask16):
            av = av_of[nt]
            for t in range(half * 4, half * 4 + 4):
                nc.tensor.matmul(av[:, t, :],
                                 lhsT=att[:, t * 128:(t + 1) * 128],
                                 rhs=v_sb[:, sk, :],
                                 start=(sk == 0 and t == 0),
                                 stop=(sk == NK - 1 and t == NSQT - 1))
            for t in range(half * 4, half * 4 + 4):
                i = state["den_idx"]
                state["den_idx"] = i + 1
                nc.tensor.matmul(den[:, nt * NSQT + t:nt * NSQT + t + 1],
                                 lhsT=att[:, t * 128:(t + 1) * 128],
                                 rhs=mask16[:, sk:sk + 1],
                                 start=(i == 0),
                                 stop=(i == NT * NK * NSQT - 1))

        def flush_av(mask16, keep=2, n_max=2, only_nt=None):
            # n_max spreads AV backlog bursts (waiting on a v slab) across
            # several attn halves so they don't clog the PE queue at once;
            # only_nt avoids parking AVs whose exp hasn't run (in-order PE
            # head-of-line) when finishing one block while the other still
            # streams
            issued = 0
            idx = 0
            while issued < n_max and idx < len(pending_av) - keep:
                att, nt, sk, half = pending_av[idx]
                if sk > state["v_ready"] or (only_nt is not None
                                             and nt != only_nt):
                    idx += 1
                    continue
                pending_av.pop(idx)
                issue_av(att, nt, sk, half, mask16)
                issued += 1

        att_of = {}

        def attn_half(nt, sk, i, ident8, mask16):
            # independent 512-column halves: 1-bank PSUM slots (deep
            # pipeline) and finer exp/AV dependencies
            sq0 = nt * SQ_BLK
            bias_t = bias_of[(nt, sk)]
            if (nt, sk) not in att_of:
                att_of[(nt, sk)] = att_pool.tile(
                    [128, SQ_BLK], F16, tag="att", name=f"att_{nt}_{sk}")
            att = att_of[(nt, sk)]
            cols = slice(i * 512, (i + 1) * 512)
            sc = ps_wk.tile([128, 512], FP, tag="wk",
                            name=f"sc_{nt}_{sk}_{i}")
            eng = _PAT[state["tile_idx"] % len(_PAT)]
            state["tile_idx"] += 1
            nc.tensor.matmul(
                sc,
                lhsT=kT_sb[:, sk * 128:(sk + 1) * 128],
                rhs=qT_sb[:, sq0 + i * 512:sq0 + (i + 1) * 512],
                start=True, stop=(eng != "pe"))
            if eng == "pe":
                nc.tensor.matmul(sc, lhsT=ident8, rhs=bias_t[:, cols],
                                 start=False, stop=True)
            else:
                nc.vector.tensor_add(out=sc, in0=sc, in1=bias_t[:, cols])
            nc.scalar.activation(out=att[:, cols], in_=sc,
                                 func=mybir.ActivationFunctionType.Exp,
                                 scale=inv_sqrt_d)
            # AV trails so the in-order PE queue never head-of-line
            # blocks on an exp result
            pending_av.append((att, nt, sk, i))
            flush_av(mask16, keep=2)

        def attn(nt, sk, ident8, mask16):
            attn_half(nt, sk, 0, ident8, mask16)
            attn_half(nt, sk, 1, ident8, mask16)

        def start_nt(nt):
            av_of[nt] = ps_av.tile([128, NSQT, D], FP, tag="av",
                                   name=f"av_{nt}")

        def finish_nt(nt, mask16):
            flush_av(mask16, keep=0, n_max=10000, only_nt=nt)
            avs = avsb_pool.tile([128, NSQT, D + 1], F16, tag="avs",
                                 name=f"avs_{nt}")
            nc.vector.tensor_copy(out=avs[:, :, 0:D], in_=av_of[nt])
            nc.vector.tensor_copy(
                out=avs[:, :, D],
                in_=den[:, nt * NSQT:(nt + 1) * NSQT])
            nc.sync.dma_start(
                out=out_d[nt], in_=avs.rearrange("p t d -> p (t d)"))

        # ---- the woven stream (DMA order is the schedule) ----
        # All x slabs are fetched as two half-slab DMAs and all bias groups
        # as 2-tile fetches, interpolated so the exp-work supply tracks the
        # ACT engine's consumption (~2.45us of exp work per 2-tile bias)
        # with no dry spells after the pipeline fills.
        fetch_w("k", wkT)
        xk0 = fetch_x("k", 0, split=True)
        fetch_w("q", wqT)
        cp, ident8, mask16 = fetch_consts()
        bq_col = cp[0:D, NK:NK + 1]
        wk_flat = w_sb["k"].rearrange("p t d -> p (t d)")
        n_warm = int(os.environ.get("KERNEL_WARMUP", "9"))
        if n_warm:
            warm = ps_wk.tile([D, 512], FP, tag="wk", name="warm")
            for i in range(n_warm):
                nc.tensor.matmul(warm, lhsT=w_sb["k"][:, 0, :], rhs=wk_flat,
                                 start=True, stop=True)
        proj_kq("k", kT_sb, 0, xk0, split_copy=True)
        xq0 = fetch_x("q", 0, split=True)
        proj_kq("q", qT_sb, 0, xq0, bq_col)
        fetch_bias(0, 0, n=2)
        fetch_bias(0, 2, n=2)
        start_nt(0)
        start_nt(1)
        for sk in range(0, 4):
            attn_half(0, sk, 0, ident8, mask16)
        xq1 = fetch_x("q", 1, split=True)
        proj_kq("q", qT_sb, 1, xq1, bq_col)
        for sk in range(0, 4):
            attn_half(0, sk, 1, ident8, mask16)
        xk1 = fetch_x("k", 1, split=True)
        proj_kq("k", kT_sb, 1, xk1, split_copy=True)
        fetch_bias(0, 4, n=2)
        attn(0, 4, ident8, mask16)
        attn(0, 5, ident8, mask16)
        fetch_bias(0, 6, n=2)
        attn(0, 6, ident8, mask16)
        attn(0, 7, ident8, mask16)
        xq2 = fetch_x("q", 2, split=True)
        proj_kq("q", qT_sb, 2, xq2, bq_col)
        fetch_bias(1, 0, n=2)
        attn_half(1, 0, 0, ident8, mask16)
        attn_half(1, 1, 0, ident8, mask16)
        fetch_bias(1, 2, n=2)
        attn_half(1, 2, 0, ident8, mask16)
        attn_half(1, 3, 0, ident8, mask16)
        xq3 = fetch_x("q", 3, split=True)
        proj_kq("q", qT_sb, 3, xq3, bq_col)
        for sk in range(0, 4):
            attn_half(1, sk, 1, ident8, mask16)
        fetch_bias(1, 4, n=2)
        attn(1, 4, ident8, mask16)
        attn(1, 5, ident8, mask16)
        fetch_bias(1, 6, n=2)
        attn(1, 6, ident8, mask16)
        attn(1, 7, ident8, mask16)
        xk2 = fetch_x("k", 2, split=True)
        proj_kq("k", kT_sb, 2, xk2, split_copy=True)
        fetch_bias(0, 8, n=2)
        attn(0, 8, ident8, mask16)
        attn(0, 9, ident8, mask16)
        fetch_bias(0, 10, n=2)
        attn(0, 10, ident8, mask16)
        attn(0, 11, ident8, mask16)
        fetch_w("v", wvT)
        xv0 = fetch_x("v", 0, split=True)
        proj_v(0, xv0, cp)
        state["v_ready"] = 3
        flush_av(mask16, keep=2, n_max=6)
        fetch_bias(1, 8, n=2)
        attn(1, 8, ident8, mask16)
        attn(1, 9, ident8, mask16)
        fetch_bias(1, 10, n=2)
        attn(1, 10, ident8, mask16)
        attn(1, 11, ident8, mask16)
        xk3 = fetch_x("k", 3, split=True)
        proj_kq("k", kT_sb, 3, xk3, split_copy=True)
        fetch_bias(0, 12, n=2)
        attn(0, 12, ident8, mask16)
        attn(0, 13, ident8, mask16)
        fetch_bias(0, 14, n=2)
        attn(0, 14, ident8, mask16)
        attn(0, 15, ident8, mask16)
        xv1 = fetch_x("v", 1, split=True)
        proj_v(1, xv1, cp)
        state["v_ready"] = 7
        flush_av(mask16, keep=2, n_max=8)
        fetch_bias(1, 12, n=2)
        attn(1, 12, ident8, mask16)
        attn(1, 13, ident8, mask16)
        xv2 = fetch_x("v", 2, split=True)
        proj_v(2, xv2, cp)
        state["v_ready"] = 11
        flush_av(mask16, keep=2, n_max=8)
        fetch_bias(1, 14, n=1)
        fetch_bias(1, 15, n=1)
        attn(1, 14, ident8, mask16)
        flush_av(mask16, keep=2, n_max=8)
        attn(1, 15, ident8, mask16)
        xv3 = fetch_x("v", 3, split=True)
        proj_v(3, xv3, cp)
        state["v_ready"] = 15
        flush_av(mask16, keep=0, n_max=100, only_nt=0)
        finish_nt(0, mask16)
        finish_nt(1, mask16)

    nc.compile()
    return nc


_NC = None


def _get_nc():
    global _NC
    if _NC is None:
        _NC = build_bass()
    return _NC


def _prep_core_inputs(b, query, key, value, relative_biases, mask,
                      Wq, bq, Wk, bk, Wv, bv):
    def wprep(W):
        # SBUF image [128, NH*D]: (p, t*D+d) = W.T[t*128+p, d]
        return np.ascontiguousarray(
            W.T.astype(np.float16).reshape(NH, 128, D).transpose(
                1, 0, 2).reshape(128, NH * D))

    cp = np.zeros((128, NK + 1), np.float32)
    cp[:, 0:NK] = mask[b].astype(np.float32).reshape(NK, 128).T
    cp[0:D, NK] = np.asarray(bq, np.float32)

    return {
        "xqT": np.ascontiguousarray(query[b].T.astype(np.float16)),
        "xkT": np.ascontiguousarray(key[b].T.astype(np.float16)),
        "xvT": np.ascontiguousarray(value[b].T.astype(np.float16)),
        "biasT": np.ascontiguousarray(
            relative_biases[b].T.astype(_np_f8())),
        "cpk": cp,
        "wqT": wprep(Wq),
        "wkT": wprep(Wk),
        "wvT": wprep(Wv),
    }


def kernel(query, key, value, relative_biases, mask, Wq, bq, Wk, bk, Wv, bv):
    query = np.asarray(query, np.float32)
    key = np.asarray(key, np.float32)
    value = np.asarray(value, np.float32)
    relative_biases = np.asarray(relative_biases, np.float32)
    mask = np.asarray(mask)
    Wq, Wk, Wv = (np.asarray(w, np.float32) for w in (Wq, Wk, Wv))
    bv_row = np.asarray(bv, np.float32).reshape(1, D)

    nc = _get_nc()
    in_maps = [
        _prep_core_inputs(b, query, key, value, relative_biases, mask,
                          Wq, bq, Wk, bk, Wv, bv)
        for b in range(B)
    ]
    res = run_bass_kernel_spmd(nc, in_maps, core_ids=list(range(N_CORES)))
    outs = []
    for i in range(N_CORES):
        o = np.asarray(res.results[i]["out"], dtype=np.float32)
        o = o.reshape(NT, 128, NSQT, D + 1)
        o = o[:, :, :, :D] / o[:, :, :, D:D + 1] + bv_row
        # out[s = nt*1024 + t*128 + p, d] = o[nt, p, t, d]
        outs.append(o.transpose(0, 2, 1, 3).reshape(S, D))
    return np.stack(outs, axis=0).astype(np.float32)
